# revision 5
# baseline (speedup 1.0000x reference)
"""ConvLSTM (reduces to plain LSTM: conv over length-1 axis -> only middle tap).

Strategy: data-parallel over batch across 8 NeuronCores (B_local = 8/core).
  Phase 1 (bulk, parallel over time): gates_x = Wx @ x + b for all steps,
          stored bf16 in DRAM, gate-major-transposed layout.
  Phase 2 (sequential scan over S=2048): per step the recurrent matmul
          Wh @ h (bf16 weights stationary, h moving, N=8), gate adds on DVE,
          sigmoid/tanh on ACT, cell math on DVE; h written bf16 to DRAM.

Layouts (per core):
  Gate rows reordered to [g, i, f, o] blocks of 512 (ref order i,f,o,g).
  M-chunk m in 0..15: reordered gate rows m*128..m*128+127 (gamma = m//4, j = m%4).
  hidden unit u = 128*q + p lives at partition p, free-slot q.
  h/c state tiles: [128, 32] with col = q*8 + b_local.
"""

import sys
import numpy as np

for _p in ("/opt/trn_rl_repo",):
    if _p not in sys.path:
        sys.path.append(_p)

import concourse.bass as bass
import concourse.mybir as mybir
from concourse.tile import TileContext
from concourse import bass_utils
from ml_dtypes import bfloat16

AF = mybir.ActivationFunctionType
FP32 = mybir.dt.float32
BF16 = mybir.dt.bfloat16

B, CIN, S, HC = 64, 256, 2048, 512
NCORES = 8
BL = B // NCORES          # 8 batch per core
G4 = 4 * HC               # 2048 gate rows
T = 128                   # steps per For_i block
NBLK = S // T
NTOK = BL * S             # 16384 tokens per core
TOKB = 512                # tokens per precompute matmul
NTB = NTOK // TOKB        # 32 token blocks
# ref gate row order [i, f, o, g]; ours [g, i, f, o]
GPERM = np.concatenate([np.arange(1536, 2048), np.arange(0, 512),
                        np.arange(512, 1024), np.arange(1024, 1536)])


def _split_multiwaits(nc):
    """This walrus build allows only ONE sync-wait command per instruction.
    Hoist extra waits onto single-wait NoOps on the same engine stream."""
    nnop = 0
    for f in nc.m.functions:
        for blk in f.blocks:
            newl = []
            dirty = False
            for inst in blk.instructions:
                si = inst.sync_info
                if si and si.on_wait and len(si.on_wait) > 1:
                    waits = list(si.on_wait)
                    for w in waits[:-1]:
                        nop = mybir.InstNoOp(name=f"wsplit-{nnop}")
                        nnop += 1
                        nop.engine = inst.engine
                        nop.sync_info = mybir.SyncInfo(on_wait=[w], on_update=[])
                        newl.append(nop)
                    inst.sync_info = mybir.SyncInfo(
                        on_wait=[waits[-1]], on_update=list(si.on_update))
                    dirty = True
                newl.append(inst)
            if dirty:
                blk.instructions = newl
    return nnop


def build_nc():
    nc = bass.Bass()
    x_d = nc.dram_tensor("x", [128, 2, S, BL], BF16, kind="ExternalInput")
    whT_d = nc.dram_tensor("whT", [128, 4, G4], BF16, kind="ExternalInput")
    wxT_d = nc.dram_tensor("wxT", [128, 2, G4], BF16, kind="ExternalInput")
    b_d = nc.dram_tensor("bias", [128, 16], FP32, kind="ExternalInput")
    gx_d = nc.dram_tensor("gx", [4, 128, S, 4, BL], BF16, kind="Internal")
    out_d = nc.dram_tensor("out", [128, S, 4, BL], BF16, kind="ExternalOutput")

    with TileContext(nc) as tc:
        with (
            tc.tile_pool(name="const", bufs=1) as cpool,
            tc.tile_pool(name="state", bufs=1) as spool,
        ):
            whT = cpool.tile([128, 4 * G4], BF16)
            wxT = cpool.tile([128, 2 * G4], BF16)
            bias = cpool.tile([128, 16], FP32)
            nc.sync.dma_start(out=whT[:, :], in_=whT_d[:, :, :])
            nc.sync.dma_start(out=wxT[:, :], in_=wxT_d[:, :, :])
            nc.sync.dma_start(out=bias[:, :], in_=b_d[:, :])

            h_st = spool.tile([128, 32], BF16)
            c_st = spool.tile([128, 32], FP32)
            nc.vector.memset(h_st[:, :], 0.0)
            nc.vector.memset(c_st[:, :], 0.0)

            # ---------------- Phase 1: gates_x precompute ----------------
            with (
                tc.tile_pool(name="xin", bufs=3) as xpool,
                tc.tile_pool(name="pcps", bufs=4, space="PSUM") as pcps,
                tc.tile_pool(name="gxe", bufs=4) as gxep,
            ):
                for tb in range(NTB):
                    t0 = tb * (TOKB // BL)  # 64 steps per token block
                    xt = [xpool.tile([128, TOKB], BF16, tag=f"x{k}", name=f"xt{k}") for k in range(2)]
                    for k in range(2):
                        nc.sync.dma_start(
                            out=xt[k][:, :],
                            in_=x_d[:, k, t0:t0 + TOKB // BL, :])
                    for m in range(16):
                        ps = pcps.tile([128, TOKB], FP32, tag="pc")
                        for k in range(2):
                            nc.tensor.matmul(
                                ps[:, :],
                                wxT[:, k * G4 + m * 128: k * G4 + (m + 1) * 128],
                                xt[k][:, :],
                                start=(k == 0), stop=(k == 1))
                        ge = gxep.tile([128, TOKB], BF16, tag="ge")
                        nc.scalar.activation(
                            out=ge[:, :], in_=ps[:, :], func=AF.Identity,
                            bias=bias[:, m:m + 1])
                        nc.sync.dma_start(
                            out=gx_d[m // 4, :, t0:t0 + TOKB // BL, m % 4, :],
                            in_=ge[:, :])

            # DRAM (gx_d) RAW across phases is not tracked by Tile -> hard barrier
            tc.strict_bb_all_engine_barrier()

            # ---------------- Phase 2: recurrence ----------------
            with (
                tc.tile_pool(name="gxin", bufs=2) as gxp,
                tc.tile_pool(name="obuf", bufs=2) as obp,
                tc.tile_pool(name="rps", bufs=2, space="PSUM") as rps,
                tc.tile_pool(name="work", bufs=3) as wk,
            ):
                with tc.For_i(0, S, T) as i0:
                    gxt = [gxp.tile([128, T * 32], BF16, tag=f"gx{g}", name=f"gxt{g}") for g in range(4)]
                    for g in range(4):
                        nc.sync.dma_start(
                            out=gxt[g][:, :],
                            in_=gx_d[g, :, bass.ds(i0, T), :, :])
                    ob = obp.tile([128, T * 32], BF16, tag="ob")
                    for t in range(T):
                        ps = [rps.tile([128, 32], FP32, tag=f"ps{g}", name=f"ps{g}") for g in range(4)]
                        for g in range(4):
                            for j in range(4):
                                m = g * 4 + j
                                for k in range(4):
                                    nc.tensor.matmul(
                                        ps[g][:, j * 8:(j + 1) * 8],
                                        whT[:, k * G4 + m * 128: k * G4 + (m + 1) * 128],
                                        h_st[:, k * 8:(k + 1) * 8],
                                        start=(k == 0), stop=(k == 3))
                        # gate order: 0=g 1=i 2=f 3=o
                        ga = [wk.tile([128, 32], FP32, tag=f"ga{g}", name=f"ga{g}") for g in range(4)]
                        ac = [wk.tile([128, 32], FP32, tag=f"ac{g}", name=f"ac{g}") for g in range(4)]
                        for g in range(4):
                            nc.vector.tensor_add(
                                out=ga[g][:, :], in0=ps[g][:, :],
                                in1=gxt[g][:, t * 32:(t + 1) * 32])
                            nc.scalar.activation(
                                out=ac[g][:, :], in_=ga[g][:, :],
                                func=(AF.Tanh if g == 0 else AF.Sigmoid))
                        ig = wk.tile([128, 32], FP32, tag="ig")
                        fc = wk.tile([128, 32], FP32, tag="fc")
                        tc_ = wk.tile([128, 32], FP32, tag="tc")
                        nc.vector.tensor_mul(out=ig[:, :], in0=ac[1][:, :], in1=ac[0][:, :])
                        nc.vector.tensor_mul(out=fc[:, :], in0=ac[2][:, :], in1=c_st[:, :])
                        nc.vector.tensor_add(out=c_st[:, :], in0=fc[:, :], in1=ig[:, :])
                        nc.scalar.activation(out=tc_[:, :], in_=c_st[:, :], func=AF.Tanh)
                        nc.vector.tensor_mul(out=h_st[:, :], in0=ac[3][:, :], in1=tc_[:, :])
                        nc.vector.tensor_copy(out=ob[:, t * 32:(t + 1) * 32], in_=h_st[:, :])
                    nc.sync.dma_start(out=out_d[:, bass.ds(i0, T), :, :], in_=ob[:, :])
    _split_multiwaits(nc)
    return nc


def _prep_core_inputs(x_core, W, b):
    """x_core [BL, 256, S] f32 -> per-core input dict."""
    Wm = W[:, :, 1][GPERM]              # [2048, 768] reordered rows
    Wx = Wm[:, :CIN]                    # [2048, 256]
    Wh = Wm[:, CIN:]                    # [2048, 512]
    whT = np.ascontiguousarray(
        Wh.T.reshape(4, 128, G4).transpose(1, 0, 2)).astype(bfloat16)
    wxT = np.ascontiguousarray(
        Wx.T.reshape(2, 128, G4).transpose(1, 0, 2)).astype(bfloat16)
    bias = np.ascontiguousarray(b[GPERM].reshape(16, 128).T).astype(np.float32)
    # x_d [128 p, 2 kc, S, BL]: x_core[b, kc*128+p, s]
    xr = np.ascontiguousarray(
        x_core.reshape(BL, 2, 128, S).transpose(2, 1, 3, 0)).astype(bfloat16)
    return {"x": xr, "whT": whT, "wxT": wxT, "bias": bias}


def kernel(x, W, b):
    x = np.asarray(x, dtype=np.float32)
    W = np.asarray(W, dtype=np.float32)
    b = np.asarray(b, dtype=np.float32)
    nc = build_nc()
    in_maps = [_prep_core_inputs(x[c * BL:(c + 1) * BL], W, b)
               for c in range(NCORES)]
    res = bass_utils.run_bass_kernel_spmd(nc, in_maps, core_ids=list(range(NCORES)))
    outs = []
    for c in range(NCORES):
        o = np.asarray(res.results[c]["out"], dtype=np.float32)  # [128, S, 4, BL]
        outs.append(o.transpose(3, 2, 0, 1).reshape(BL, HC, S))
    return np.concatenate(outs, axis=0)


if __name__ == "__main__":
    d = np.load("/root/problem/ref_cache.npz")
    out = kernel(d["x"], d["W"], d["b"])
    exp = d["expected"]
    err = np.abs(out - exp).max() / (np.abs(exp).max() + 1e-9)
    print("rel err:", err)


# revision 9
# speedup vs baseline: 2.5579x; 2.5579x over previous
"""ConvLSTM (reduces to plain LSTM: conv over length-1 axis -> only middle tap).

Strategy: data-parallel over batch across 8 NeuronCores (B_local = 8/core).
  Phase 1 (bulk, parallel over time): gates_x = Wx @ x + b for all steps,
          stored bf16 in DRAM, gate-major-transposed layout.
  Phase 2 (sequential scan over S=2048): per step the recurrent matmul
          Wh @ h (bf16 weights stationary, h moving, N=8), gate adds on DVE,
          sigmoid/tanh on ACT, cell math on DVE; h written bf16 to DRAM.

Layouts (per core):
  Gate rows reordered to [g, i, f, o] blocks of 512 (ref order i,f,o,g).
  M-chunk m in 0..15: reordered gate rows m*128..m*128+127 (gamma = m//4, j = m%4).
  hidden unit u = 128*q + p lives at partition p, free-slot q.
  h/c state tiles: [128, 32] with col = q*8 + b_local.
"""

import sys
import numpy as np

for _p in ("/opt/trn_rl_repo",):
    if _p not in sys.path:
        sys.path.append(_p)

import concourse.bass as bass
import concourse.mybir as mybir
from concourse.tile import TileContext
from concourse import bass_utils
from ml_dtypes import bfloat16

AF = mybir.ActivationFunctionType
FP32 = mybir.dt.float32
BF16 = mybir.dt.bfloat16

B, CIN, S, HC = 64, 256, 2048, 512
NCORES = 8
BL = B // NCORES          # 8 batch per core
G4 = 4 * HC               # 2048 gate rows
T = 128                   # steps per For_i block
NBLK = S // T
NTOK = BL * S             # 16384 tokens per core
TOKB = 512                # tokens per precompute matmul
NTB = NTOK // TOKB        # 32 token blocks
# ref gate row order [i, f, o, g]; ours [g, i, f, o]
GPERM = np.concatenate([np.arange(1536, 2048), np.arange(0, 512),
                        np.arange(512, 1024), np.arange(1024, 1536)])


_KLDWOPT = True


def _patch_walrus_flags():
    """Enable walrus LDW optimization (background weight buffer) - requires
    self-loading matmuls (no standalone InstLdweights)."""
    if not _KLDWOPT:
        return
    import concourse.bass_utils as _bu
    if getattr(_bu.run_command, "_ldwopt_patched", False):
        return
    _orig = _bu.run_command

    def _run(cmd, **kw):
        cmd = ["--enable-ldw-opt=true" if c == "--enable-ldw-opt=false" else c
               for c in cmd]
        return _orig(cmd, **kw)

    _run._ldwopt_patched = True
    _bu.run_command = _run


def _refuse_ldweights(nc):
    """Fold each standalone InstLdweights into its following InstMatmult
    (self-loading matmul), merging sync waits/updates."""
    for f in nc.m.functions:
        for blk in f.blocks:
            newl = []
            pending = None
            for inst in blk.instructions:
                tn = type(inst).__name__
                if tn == "InstLdweights":
                    assert pending is None
                    pending = inst
                    continue
                if tn == "InstMatmult" and pending is not None:
                    lw = list(pending.sync_info.on_wait) if pending.sync_info else []
                    lu = list(pending.sync_info.on_update) if pending.sync_info else []
                    mw = list(inst.sync_info.on_wait) if inst.sync_info else []
                    mu = list(inst.sync_info.on_update) if inst.sync_info else []
                    inst.sync_info = mybir.SyncInfo(on_wait=lw + mw, on_update=lu + mu)
                    inst.ldweights = True
                    pending = None
                newl.append(inst)
            assert pending is None, "trailing InstLdweights without matmul"
            blk.instructions = newl


def _split_multiwaits(nc):
    """This walrus build allows only ONE sync-wait command per instruction.
    Hoist extra waits onto single-wait NoOps on the same engine stream."""
    nnop = 0
    for f in nc.m.functions:
        for blk in f.blocks:
            newl = []
            dirty = False
            for inst in blk.instructions:
                si = inst.sync_info
                if si and si.on_wait and len(si.on_wait) > 1:
                    waits = list(si.on_wait)
                    for w in waits[:-1]:
                        nop = mybir.InstNoOp(name=f"wsplit-{nnop}")
                        nnop += 1
                        nop.engine = inst.engine
                        nop.sync_info = mybir.SyncInfo(on_wait=[w], on_update=[])
                        newl.append(nop)
                    inst.sync_info = mybir.SyncInfo(
                        on_wait=[waits[-1]], on_update=list(si.on_update))
                    dirty = True
                newl.append(inst)
            if dirty:
                blk.instructions = newl
    return nnop


def build_nc():
    nc = bass.Bass()
    x_d = nc.dram_tensor("x", [128, 2, S, BL], BF16, kind="ExternalInput")
    whT_d = nc.dram_tensor("whT", [128, 4, G4], BF16, kind="ExternalInput")
    wxT_d = nc.dram_tensor("wxT", [128, 2, G4], BF16, kind="ExternalInput")
    b_d = nc.dram_tensor("bias", [128, 16], FP32, kind="ExternalInput")
    gx_d = nc.dram_tensor("gx", [4, 128, S, 4, BL], BF16, kind="Internal")
    out_d = nc.dram_tensor("out", [128, S, 4, BL], BF16, kind="ExternalOutput")

    with TileContext(nc) as tc:
        with (
            tc.tile_pool(name="const", bufs=1) as cpool,
            tc.tile_pool(name="state", bufs=1) as spool,
        ):
            whT = cpool.tile([128, 4 * G4], BF16)
            wxT = cpool.tile([128, 2 * G4], BF16)
            bias = cpool.tile([128, 16], FP32)
            nc.sync.dma_start(out=whT[:, :], in_=whT_d[:, :, :])
            nc.sync.dma_start(out=wxT[:, :], in_=wxT_d[:, :, :])
            nc.sync.dma_start(out=bias[:, :], in_=b_d[:, :])

            h_st = spool.tile([128, 32], BF16)
            c_st = spool.tile([128, 32], FP32)
            nc.vector.memset(h_st[:, :], 0.0)
            nc.vector.memset(c_st[:, :], 0.0)

            # ---------------- Phase 1: gates_x precompute ----------------
            with (
                tc.tile_pool(name="xin", bufs=3) as xpool,
                tc.tile_pool(name="pcps", bufs=4, space="PSUM") as pcps,
                tc.tile_pool(name="gxe", bufs=4) as gxep,
            ):
                TS = TOKB // BL  # 64 steps per token block
                for tb in range(NTB):
                    t0 = tb * TS
                    xt = [xpool.tile([128, TOKB], BF16, tag=f"x{k}", name=f"xt{k}") for k in range(2)]
                    for k in range(2):
                        nc.sync.dma_start(
                            out=xt[k][:, :],
                            in_=x_d[:, k, t0:t0 + TS, :])
                    for g in range(4):
                        # stage all 4 j-chunks of gate g in (t, j, b) order so
                        # the DRAM write is one fully-contiguous burst per row
                        ge = gxep.tile([128, 4 * TOKB], BF16, tag="ge")
                        gev = ge.rearrange("p (t j b) -> p t j b", t=TS, j=4, b=BL)
                        for j in range(4):
                            m = g * 4 + j
                            ps = pcps.tile([128, TOKB], FP32, tag="pc")
                            for k in range(2):
                                nc.tensor.matmul(
                                    ps[:, :],
                                    wxT[:, k * G4 + m * 128: k * G4 + (m + 1) * 128],
                                    xt[k][:, :],
                                    start=(k == 0), stop=(k == 1))
                            nc.scalar.activation(
                                out=gev[:, :, j, :], in_=ps[:, :], func=AF.Identity,
                                bias=bias[:, m:m + 1])
                        nc.gpsimd.dma_start(
                            out=gx_d[g, :, t0:t0 + TS, :, :],
                            in_=ge[:, :])

            # DRAM (gx_d) RAW across phases is not tracked by Tile -> hard barrier
            tc.strict_bb_all_engine_barrier()

            # ---------------- Phase 2: recurrence ----------------
            with (
                tc.tile_pool(name="gxin", bufs=2) as gxp,
                tc.tile_pool(name="obuf", bufs=2) as obp,
                tc.tile_pool(name="rps", bufs=2, space="PSUM") as rps,
                tc.tile_pool(name="work", bufs=3) as wk,
            ):
                with tc.For_i(0, S, T) as i0:
                    gxt = [gxp.tile([128, T * 32], BF16, tag=f"gx{g}", name=f"gxt{g}") for g in range(4)]
                    for g in range(4):
                        nc.sync.dma_start(
                            out=gxt[g][:, :],
                            in_=gx_d[g, :, bass.ds(i0, T), :, :])
                    ob = obp.tile([128, T * 32], BF16, tag="ob")
                    for t in range(T):
                        ps = [rps.tile([128, 32], FP32, tag=f"ps{g}", name=f"ps{g}") for g in range(4)]
                        for g in range(4):
                            for j in range(4):
                                m = g * 4 + j
                                for k in range(4):
                                    nc.tensor.matmul(
                                        ps[g][:, j * 8:(j + 1) * 8],
                                        whT[:, k * G4 + m * 128: k * G4 + (m + 1) * 128],
                                        h_st[:, k * 8:(k + 1) * 8],
                                        start=(k == 0), stop=(k == 3))
                        # gate order: 0=g 1=i 2=f 3=o
                        ga = [wk.tile([128, 32], FP32, tag=f"ga{g}", name=f"ga{g}") for g in range(4)]
                        ac = [wk.tile([128, 32], FP32, tag=f"ac{g}", name=f"ac{g}") for g in range(4)]
                        for g in range(4):
                            nc.vector.tensor_add(
                                out=ga[g][:, :], in0=ps[g][:, :],
                                in1=gxt[g][:, t * 32:(t + 1) * 32])
                            nc.scalar.activation(
                                out=ac[g][:, :], in_=ga[g][:, :],
                                func=(AF.Tanh if g == 0 else AF.Sigmoid))
                        ig = wk.tile([128, 32], FP32, tag="ig")
                        fc = wk.tile([128, 32], FP32, tag="fc")
                        tc_ = wk.tile([128, 32], FP32, tag="tc")
                        nc.vector.tensor_mul(out=ig[:, :], in0=ac[1][:, :], in1=ac[0][:, :])
                        nc.vector.tensor_mul(out=fc[:, :], in0=ac[2][:, :], in1=c_st[:, :])
                        nc.vector.tensor_add(out=c_st[:, :], in0=fc[:, :], in1=ig[:, :])
                        nc.scalar.activation(out=tc_[:, :], in_=c_st[:, :], func=AF.Tanh)
                        nc.vector.tensor_mul(out=h_st[:, :], in0=ac[3][:, :], in1=tc_[:, :])
                        nc.vector.tensor_copy(out=ob[:, t * 32:(t + 1) * 32], in_=h_st[:, :])
                    nc.sync.dma_start(out=out_d[:, bass.ds(i0, T), :, :], in_=ob[:, :])
    if _KLDWOPT:
        _patch_walrus_flags()
        _refuse_ldweights(nc)
    _split_multiwaits(nc)
    return nc


def _prep_core_inputs(x_core, W, b):
    """x_core [BL, 256, S] f32 -> per-core input dict."""
    Wm = W[:, :, 1][GPERM]              # [2048, 768] reordered rows
    Wx = Wm[:, :CIN]                    # [2048, 256]
    Wh = Wm[:, CIN:]                    # [2048, 512]
    whT = np.ascontiguousarray(
        Wh.T.reshape(4, 128, G4).transpose(1, 0, 2)).astype(bfloat16)
    wxT = np.ascontiguousarray(
        Wx.T.reshape(2, 128, G4).transpose(1, 0, 2)).astype(bfloat16)
    bias = np.ascontiguousarray(b[GPERM].reshape(16, 128).T).astype(np.float32)
    # x_d [128 p, 2 kc, S, BL]: x_core[b, kc*128+p, s]
    xr = np.ascontiguousarray(
        x_core.reshape(BL, 2, 128, S).transpose(2, 1, 3, 0)).astype(bfloat16)
    return {"x": xr, "whT": whT, "wxT": wxT, "bias": bias}


def kernel(x, W, b):
    x = np.asarray(x, dtype=np.float32)
    W = np.asarray(W, dtype=np.float32)
    b = np.asarray(b, dtype=np.float32)
    nc = build_nc()
    in_maps = [_prep_core_inputs(x[c * BL:(c + 1) * BL], W, b)
               for c in range(NCORES)]
    res = bass_utils.run_bass_kernel_spmd(nc, in_maps, core_ids=list(range(NCORES)))
    outs = []
    for c in range(NCORES):
        o = np.asarray(res.results[c]["out"], dtype=np.float32)  # [128, S, 4, BL]
        outs.append(o.transpose(3, 2, 0, 1).reshape(BL, HC, S))
    return np.concatenate(outs, axis=0)


if __name__ == "__main__":
    d = np.load("/root/problem/ref_cache.npz")
    out = kernel(d["x"], d["W"], d["b"])
    exp = d["expected"]
    err = np.abs(out - exp).max() / (np.abs(exp).max() + 1e-9)
    print("rel err:", err)


# revision 10
# speedup vs baseline: 2.5757x; 1.0069x over previous
"""ConvLSTM (reduces to plain LSTM: conv over length-1 axis -> only middle tap).

Strategy: data-parallel over batch across 8 NeuronCores (B_local = 8/core).
  Phase 1 (bulk, parallel over time): gates_x = Wx @ x + b for all steps,
          stored bf16 in DRAM, gate-major-transposed layout.
  Phase 2 (sequential scan over S=2048): per step the recurrent matmul
          Wh @ h (bf16 weights stationary, h moving, N=8), gate adds on DVE,
          sigmoid/tanh on ACT, cell math on DVE; h written bf16 to DRAM.

Layouts (per core):
  Gate rows reordered to [g, i, f, o] blocks of 512 (ref order i,f,o,g).
  M-chunk m in 0..15: reordered gate rows m*128..m*128+127 (gamma = m//4, j = m%4).
  hidden unit u = 128*q + p lives at partition p, free-slot q.
  h/c state tiles: [128, 32] with col = q*8 + b_local.
"""

import sys
import numpy as np

for _p in ("/opt/trn_rl_repo",):
    if _p not in sys.path:
        sys.path.append(_p)

import concourse.bass as bass
import concourse.mybir as mybir
from concourse.tile import TileContext
from concourse import bass_utils
from ml_dtypes import bfloat16

AF = mybir.ActivationFunctionType
FP32 = mybir.dt.float32
BF16 = mybir.dt.bfloat16

B, CIN, S, HC = 64, 256, 2048, 512
NCORES = 8
BL = B // NCORES          # 8 batch per core
G4 = 4 * HC               # 2048 gate rows
T = 128                   # steps per For_i block
NBLK = S // T
NTOK = BL * S             # 16384 tokens per core
TOKB = 512                # tokens per precompute matmul
NTB = NTOK // TOKB        # 32 token blocks
# ref gate row order [i, f, o, g]; ours [g, i, f, o]
GPERM = np.concatenate([np.arange(1536, 2048), np.arange(0, 512),
                        np.arange(512, 1024), np.arange(1024, 1536)])


_KLDWOPT = False


def _patch_walrus_flags():
    """Enable walrus LDW optimization (background weight buffer) - requires
    self-loading matmuls (no standalone InstLdweights)."""
    if not _KLDWOPT:
        return
    import concourse.bass_utils as _bu
    if getattr(_bu.run_command, "_ldwopt_patched", False):
        return
    _orig = _bu.run_command

    def _run(cmd, **kw):
        cmd = ["--enable-ldw-opt=true" if c == "--enable-ldw-opt=false" else c
               for c in cmd]
        return _orig(cmd, **kw)

    _run._ldwopt_patched = True
    _bu.run_command = _run


def _refuse_ldweights(nc):
    """Fold each standalone InstLdweights into its following InstMatmult
    (self-loading matmul), merging sync waits/updates."""
    for f in nc.m.functions:
        for blk in f.blocks:
            newl = []
            pending = None
            for inst in blk.instructions:
                tn = type(inst).__name__
                if tn == "InstLdweights":
                    assert pending is None
                    pending = inst
                    continue
                if tn == "InstMatmult" and pending is not None:
                    lw = list(pending.sync_info.on_wait) if pending.sync_info else []
                    lu = list(pending.sync_info.on_update) if pending.sync_info else []
                    mw = list(inst.sync_info.on_wait) if inst.sync_info else []
                    mu = list(inst.sync_info.on_update) if inst.sync_info else []
                    inst.sync_info = mybir.SyncInfo(on_wait=lw + mw, on_update=lu + mu)
                    inst.ldweights = True
                    pending = None
                newl.append(inst)
            assert pending is None, "trailing InstLdweights without matmul"
            blk.instructions = newl


def _split_multiwaits(nc):
    """This walrus build allows only ONE sync-wait command per instruction.
    Hoist extra waits onto single-wait NoOps on the same engine stream."""
    nnop = 0
    for f in nc.m.functions:
        for blk in f.blocks:
            newl = []
            dirty = False
            for inst in blk.instructions:
                si = inst.sync_info
                if si and si.on_wait and len(si.on_wait) > 1:
                    waits = list(si.on_wait)
                    for w in waits[:-1]:
                        nop = mybir.InstNoOp(name=f"wsplit-{nnop}")
                        nnop += 1
                        nop.engine = inst.engine
                        nop.sync_info = mybir.SyncInfo(on_wait=[w], on_update=[])
                        newl.append(nop)
                    inst.sync_info = mybir.SyncInfo(
                        on_wait=[waits[-1]], on_update=list(si.on_update))
                    dirty = True
                newl.append(inst)
            if dirty:
                blk.instructions = newl
    return nnop


def build_nc():
    nc = bass.Bass()
    x_d = nc.dram_tensor("x", [128, 2, S, BL], BF16, kind="ExternalInput")
    whT_d = nc.dram_tensor("whT", [128, 4, G4], BF16, kind="ExternalInput")
    wxT_d = nc.dram_tensor("wxT", [128, 2, G4], BF16, kind="ExternalInput")
    b_d = nc.dram_tensor("bias", [128, 16], FP32, kind="ExternalInput")
    gx_d = nc.dram_tensor("gx", [4, 128, S, 4, BL], BF16, kind="Internal")
    out_d = nc.dram_tensor("out", [128, S, 4, BL], BF16, kind="ExternalOutput")

    with TileContext(nc) as tc:
        with (
            tc.tile_pool(name="const", bufs=1) as cpool,
            tc.tile_pool(name="state", bufs=1) as spool,
        ):
            whT = cpool.tile([128, 4 * G4], BF16)
            wxT = cpool.tile([128, 2 * G4], BF16)
            bias = cpool.tile([128, 16], FP32)
            nc.sync.dma_start(out=whT[:, :], in_=whT_d[:, :, :])
            nc.sync.dma_start(out=wxT[:, :], in_=wxT_d[:, :, :])
            nc.sync.dma_start(out=bias[:, :], in_=b_d[:, :])

            h_st = spool.tile([128, 32], BF16)
            c_st = spool.tile([128, 32], FP32)
            nc.vector.memset(h_st[:, :], 0.0)
            nc.vector.memset(c_st[:, :], 0.0)

            # ---------------- Phase 1: gates_x precompute ----------------
            with (
                tc.tile_pool(name="xin", bufs=3) as xpool,
                tc.tile_pool(name="pcps", bufs=4, space="PSUM") as pcps,
                tc.tile_pool(name="gxe", bufs=4) as gxep,
            ):
                TS = TOKB // BL  # 64 steps per token block
                for tb in range(NTB):
                    t0 = tb * TS
                    xt = [xpool.tile([128, TOKB], BF16, tag=f"x{k}", name=f"xt{k}") for k in range(2)]
                    for k in range(2):
                        nc.sync.dma_start(
                            out=xt[k][:, :],
                            in_=x_d[:, k, t0:t0 + TS, :])
                    for g in range(4):
                        # stage all 4 j-chunks of gate g in (t, j, b) order so
                        # the DRAM write is one fully-contiguous burst per row
                        ge = gxep.tile([128, 4 * TOKB], BF16, tag="ge")
                        gev = ge.rearrange("p (t j b) -> p t j b", t=TS, j=4, b=BL)
                        for j in range(4):
                            m = g * 4 + j
                            ps = pcps.tile([128, TOKB], FP32, tag="pc")
                            for k in range(2):
                                nc.tensor.matmul(
                                    ps[:, :],
                                    wxT[:, k * G4 + m * 128: k * G4 + (m + 1) * 128],
                                    xt[k][:, :],
                                    start=(k == 0), stop=(k == 1))
                            nc.scalar.activation(
                                out=gev[:, :, j, :], in_=ps[:, :], func=AF.Identity,
                                bias=bias[:, m:m + 1])
                        nc.gpsimd.dma_start(
                            out=gx_d[g, :, t0:t0 + TS, :, :],
                            in_=ge[:, :])

            # DRAM (gx_d) RAW across phases is not tracked by Tile -> hard barrier
            tc.strict_bb_all_engine_barrier()

            # ---------------- Phase 2: recurrence ----------------
            with (
                tc.tile_pool(name="gxin", bufs=2) as gxp,
                tc.tile_pool(name="obuf", bufs=2) as obp,
                tc.tile_pool(name="rps", bufs=2, space="PSUM") as rps,
                tc.tile_pool(name="work", bufs=3) as wk,
            ):
                with tc.For_i(0, S, T) as i0:
                    gxt = [gxp.tile([128, T * 32], BF16, tag=f"gx{g}", name=f"gxt{g}") for g in range(4)]
                    for g in range(4):
                        nc.sync.dma_start(
                            out=gxt[g][:, :],
                            in_=gx_d[g, :, bass.ds(i0, T), :, :])
                    ob = obp.tile([128, T * 32], BF16, tag="ob")
                    for t in range(T):
                        ps = [rps.tile([128, 32], FP32, tag=f"ps{g}", name=f"ps{g}") for g in range(4)]
                        for g in range(4):
                            for j in range(4):
                                m = g * 4 + j
                                for k in range(4):
                                    nc.tensor.matmul(
                                        ps[g][:, j * 8:(j + 1) * 8],
                                        whT[:, k * G4 + m * 128: k * G4 + (m + 1) * 128],
                                        h_st[:, k * 8:(k + 1) * 8],
                                        start=(k == 0), stop=(k == 3))
                        # gate order: 0=g 1=i 2=f 3=o ; i&f share a tile so
                        # one ACT sigmoid covers both
                        gag = wk.tile([128, 32], FP32, tag="gag")
                        gaif = wk.tile([128, 64], FP32, tag="gaif")
                        gao = wk.tile([128, 32], FP32, tag="gao")
                        acg = wk.tile([128, 32], FP32, tag="acg")
                        acif = wk.tile([128, 64], FP32, tag="acif")
                        aco = wk.tile([128, 32], FP32, tag="aco")
                        nc.vector.tensor_add(
                            out=gag[:, :], in0=ps[0][:, :],
                            in1=gxt[0][:, t * 32:(t + 1) * 32])
                        nc.scalar.activation(out=acg[:, :], in_=gag[:, :], func=AF.Tanh)
                        nc.vector.tensor_add(
                            out=gaif[:, 0:32], in0=ps[1][:, :],
                            in1=gxt[1][:, t * 32:(t + 1) * 32])
                        nc.vector.tensor_add(
                            out=gaif[:, 32:64], in0=ps[2][:, :],
                            in1=gxt[2][:, t * 32:(t + 1) * 32])
                        nc.scalar.activation(out=acif[:, :], in_=gaif[:, :], func=AF.Sigmoid)
                        nc.vector.tensor_add(
                            out=gao[:, :], in0=ps[3][:, :],
                            in1=gxt[3][:, t * 32:(t + 1) * 32])
                        nc.scalar.activation(out=aco[:, :], in_=gao[:, :], func=AF.Sigmoid)
                        ig = wk.tile([128, 32], FP32, tag="ig")
                        fc = wk.tile([128, 32], FP32, tag="fc")
                        tc_ = wk.tile([128, 32], FP32, tag="tc")
                        nc.vector.tensor_mul(out=ig[:, :], in0=acif[:, 0:32], in1=acg[:, :])
                        nc.vector.tensor_mul(out=fc[:, :], in0=acif[:, 32:64], in1=c_st[:, :])
                        nc.vector.tensor_add(out=c_st[:, :], in0=fc[:, :], in1=ig[:, :])
                        nc.scalar.activation(out=tc_[:, :], in_=c_st[:, :], func=AF.Tanh)
                        nc.vector.tensor_mul(out=h_st[:, :], in0=aco[:, :], in1=tc_[:, :])
                        nc.gpsimd.tensor_copy(out=ob[:, t * 32:(t + 1) * 32], in_=h_st[:, :])
                    nc.sync.dma_start(out=out_d[:, bass.ds(i0, T), :, :], in_=ob[:, :])
    if _KLDWOPT:
        _patch_walrus_flags()
        _refuse_ldweights(nc)
    _split_multiwaits(nc)
    return nc


def _prep_core_inputs(x_core, W, b):
    """x_core [BL, 256, S] f32 -> per-core input dict."""
    Wm = W[:, :, 1][GPERM]              # [2048, 768] reordered rows
    Wx = Wm[:, :CIN]                    # [2048, 256]
    Wh = Wm[:, CIN:]                    # [2048, 512]
    whT = np.ascontiguousarray(
        Wh.T.reshape(4, 128, G4).transpose(1, 0, 2)).astype(bfloat16)
    wxT = np.ascontiguousarray(
        Wx.T.reshape(2, 128, G4).transpose(1, 0, 2)).astype(bfloat16)
    bias = np.ascontiguousarray(b[GPERM].reshape(16, 128).T).astype(np.float32)
    # x_d [128 p, 2 kc, S, BL]: x_core[b, kc*128+p, s]
    xr = np.ascontiguousarray(
        x_core.reshape(BL, 2, 128, S).transpose(2, 1, 3, 0)).astype(bfloat16)
    return {"x": xr, "whT": whT, "wxT": wxT, "bias": bias}


def kernel(x, W, b):
    x = np.asarray(x, dtype=np.float32)
    W = np.asarray(W, dtype=np.float32)
    b = np.asarray(b, dtype=np.float32)
    nc = build_nc()
    in_maps = [_prep_core_inputs(x[c * BL:(c + 1) * BL], W, b)
               for c in range(NCORES)]
    res = bass_utils.run_bass_kernel_spmd(nc, in_maps, core_ids=list(range(NCORES)))
    outs = []
    for c in range(NCORES):
        o = np.asarray(res.results[c]["out"], dtype=np.float32)  # [128, S, 4, BL]
        outs.append(o.transpose(3, 2, 0, 1).reshape(BL, HC, S))
    return np.concatenate(outs, axis=0)


if __name__ == "__main__":
    d = np.load("/root/problem/ref_cache.npz")
    out = kernel(d["x"], d["W"], d["b"])
    exp = d["expected"]
    err = np.abs(out - exp).max() / (np.abs(exp).max() + 1e-9)
    print("rel err:", err)


# revision 11
# speedup vs baseline: 2.5769x; 1.0005x over previous
"""ConvLSTM (reduces to plain LSTM: conv over length-1 axis -> only middle tap).

Strategy: data-parallel over batch across 8 NeuronCores (B_local = 8/core).
  Phase 1 (bulk, parallel over time): gates_x = Wx @ x + b for all steps,
          stored bf16 in DRAM, gate-major-transposed layout.
  Phase 2 (sequential scan over S=2048): per step the recurrent matmul
          Wh @ h (bf16 weights stationary, h moving, N=8), gate adds on DVE,
          sigmoid/tanh on ACT, cell math on DVE; h written bf16 to DRAM.

Layouts (per core):
  Gate rows reordered to [g, i, f, o] blocks of 512 (ref order i,f,o,g).
  M-chunk m in 0..15: reordered gate rows m*128..m*128+127 (gamma = m//4, j = m%4).
  hidden unit u = 128*q + p lives at partition p, free-slot q.
  h/c state tiles: [128, 32] with col = q*8 + b_local.
"""

import sys
import numpy as np

for _p in ("/opt/trn_rl_repo",):
    if _p not in sys.path:
        sys.path.append(_p)

import concourse.bass as bass
import concourse.mybir as mybir
from concourse.tile import TileContext
from concourse import bass_utils
from ml_dtypes import bfloat16

AF = mybir.ActivationFunctionType
FP32 = mybir.dt.float32
BF16 = mybir.dt.bfloat16

B, CIN, S, HC = 64, 256, 2048, 512
NCORES = 8
BL = B // NCORES          # 8 batch per core
G4 = 4 * HC               # 2048 gate rows
T = 128                   # steps per For_i block
NBLK = S // T
NTOK = BL * S             # 16384 tokens per core
TOKB = 512                # tokens per precompute matmul
NTB = NTOK // TOKB        # 32 token blocks
# ref gate row order [i, f, o, g]; ours [g, i, f, o]
GPERM = np.concatenate([np.arange(1536, 2048), np.arange(0, 512),
                        np.arange(512, 1024), np.arange(1024, 1536)])


_KLDWOPT = False


def _patch_walrus_flags():
    """Enable walrus LDW optimization (background weight buffer) - requires
    self-loading matmuls (no standalone InstLdweights)."""
    if not _KLDWOPT:
        return
    import concourse.bass_utils as _bu
    if getattr(_bu.run_command, "_ldwopt_patched", False):
        return
    _orig = _bu.run_command

    def _run(cmd, **kw):
        cmd = ["--enable-ldw-opt=true" if c == "--enable-ldw-opt=false" else c
               for c in cmd]
        return _orig(cmd, **kw)

    _run._ldwopt_patched = True
    _bu.run_command = _run


def _refuse_ldweights(nc):
    """Fold each standalone InstLdweights into its following InstMatmult
    (self-loading matmul), merging sync waits/updates."""
    for f in nc.m.functions:
        for blk in f.blocks:
            newl = []
            pending = None
            for inst in blk.instructions:
                tn = type(inst).__name__
                if tn == "InstLdweights":
                    assert pending is None
                    pending = inst
                    continue
                if tn == "InstMatmult" and pending is not None:
                    lw = list(pending.sync_info.on_wait) if pending.sync_info else []
                    lu = list(pending.sync_info.on_update) if pending.sync_info else []
                    mw = list(inst.sync_info.on_wait) if inst.sync_info else []
                    mu = list(inst.sync_info.on_update) if inst.sync_info else []
                    inst.sync_info = mybir.SyncInfo(on_wait=lw + mw, on_update=lu + mu)
                    inst.ldweights = True
                    pending = None
                newl.append(inst)
            assert pending is None, "trailing InstLdweights without matmul"
            blk.instructions = newl


def _split_multiwaits(nc):
    """This walrus build allows only ONE sync-wait command per instruction.
    Hoist extra waits onto single-wait NoOps on the same engine stream."""
    nnop = 0
    for f in nc.m.functions:
        for blk in f.blocks:
            newl = []
            dirty = False
            for inst in blk.instructions:
                si = inst.sync_info
                if si and si.on_wait and len(si.on_wait) > 1:
                    waits = list(si.on_wait)
                    for w in waits[:-1]:
                        nop = mybir.InstNoOp(name=f"wsplit-{nnop}")
                        nnop += 1
                        nop.engine = inst.engine
                        nop.sync_info = mybir.SyncInfo(on_wait=[w], on_update=[])
                        newl.append(nop)
                    inst.sync_info = mybir.SyncInfo(
                        on_wait=[waits[-1]], on_update=list(si.on_update))
                    dirty = True
                newl.append(inst)
            if dirty:
                blk.instructions = newl
    return nnop


def build_nc():
    nc = bass.Bass()
    x_d = nc.dram_tensor("x", [128, 2, S, BL], BF16, kind="ExternalInput")
    whT_d = nc.dram_tensor("whT", [128, 4, G4], BF16, kind="ExternalInput")
    wxT_d = nc.dram_tensor("wxT", [128, 2, G4], BF16, kind="ExternalInput")
    b_d = nc.dram_tensor("bias", [128, 16], FP32, kind="ExternalInput")
    gx_d = nc.dram_tensor("gx", [4, 128, S, 4, BL], BF16, kind="Internal")
    out_d = nc.dram_tensor("out", [128, S, 4, BL], BF16, kind="ExternalOutput")

    with TileContext(nc) as tc:
        with (
            tc.tile_pool(name="const", bufs=1) as cpool,
            tc.tile_pool(name="state", bufs=1) as spool,
        ):
            whT = cpool.tile([128, 4 * G4], BF16)
            wxT = cpool.tile([128, 2 * G4], BF16)
            bias = cpool.tile([128, 16], FP32)
            nc.sync.dma_start(out=whT[:, :], in_=whT_d[:, :, :])
            nc.sync.dma_start(out=wxT[:, :], in_=wxT_d[:, :, :])
            nc.sync.dma_start(out=bias[:, :], in_=b_d[:, :])

            h_st = spool.tile([128, 32], BF16)
            c_st = spool.tile([128, 32], FP32)
            nc.vector.memset(h_st[:, :], 0.0)
            nc.vector.memset(c_st[:, :], 0.0)

            # ---------------- Phase 1: gates_x precompute ----------------
            with (
                tc.tile_pool(name="xin", bufs=3) as xpool,
                tc.tile_pool(name="pcps", bufs=4, space="PSUM") as pcps,
                tc.tile_pool(name="gxe", bufs=4) as gxep,
            ):
                TS = TOKB // BL  # 64 steps per token block
                for tb in range(NTB):
                    t0 = tb * TS
                    xt = [xpool.tile([128, TOKB], BF16, tag=f"x{k}", name=f"xt{k}") for k in range(2)]
                    for k in range(2):
                        nc.sync.dma_start(
                            out=xt[k][:, :],
                            in_=x_d[:, k, t0:t0 + TS, :])
                    for g in range(4):
                        # stage all 4 j-chunks of gate g in (t, j, b) order so
                        # the DRAM write is one fully-contiguous burst per row
                        ge = gxep.tile([128, 4 * TOKB], BF16, tag="ge")
                        gev = ge.rearrange("p (t j b) -> p t j b", t=TS, j=4, b=BL)
                        for j in range(4):
                            m = g * 4 + j
                            ps = pcps.tile([128, TOKB], FP32, tag="pc")
                            for k in range(2):
                                nc.tensor.matmul(
                                    ps[:, :],
                                    wxT[:, k * G4 + m * 128: k * G4 + (m + 1) * 128],
                                    xt[k][:, :],
                                    start=(k == 0), stop=(k == 1))
                            nc.scalar.activation(
                                out=gev[:, :, j, :], in_=ps[:, :], func=AF.Identity,
                                bias=bias[:, m:m + 1])
                        nc.gpsimd.dma_start(
                            out=gx_d[g, :, t0:t0 + TS, :, :],
                            in_=ge[:, :])

            # DRAM (gx_d) RAW across phases is not tracked by Tile -> hard barrier
            tc.strict_bb_all_engine_barrier()

            # ---------------- Phase 2: recurrence ----------------
            with (
                tc.tile_pool(name="gxin", bufs=2) as gxp,
                tc.tile_pool(name="obuf", bufs=2) as obp,
                tc.tile_pool(name="rps", bufs=2, space="PSUM") as rps,
                tc.tile_pool(name="work", bufs=6) as wk,
            ):
                with tc.For_i(0, S, T) as i0:
                    gxt = [gxp.tile([128, T * 32], BF16, tag=f"gx{g}", name=f"gxt{g}") for g in range(4)]
                    for g in range(4):
                        nc.sync.dma_start(
                            out=gxt[g][:, :],
                            in_=gx_d[g, :, bass.ds(i0, T), :, :])
                    ob = obp.tile([128, T * 32], BF16, tag="ob")
                    for t in range(T):
                        ps = [rps.tile([128, 32], FP32, tag=f"ps{g}", name=f"ps{g}") for g in range(4)]
                        for g in range(4):
                            for j in range(4):
                                m = g * 4 + j
                                for k in range(4):
                                    nc.tensor.matmul(
                                        ps[g][:, j * 8:(j + 1) * 8],
                                        whT[:, k * G4 + m * 128: k * G4 + (m + 1) * 128],
                                        h_st[:, k * 8:(k + 1) * 8],
                                        start=(k == 0), stop=(k == 3))
                        # gate order: 0=g 1=i 2=f 3=o ; i&f share a tile so
                        # one ACT sigmoid covers both
                        gag = wk.tile([128, 32], FP32, tag="gag")
                        gaif = wk.tile([128, 64], FP32, tag="gaif")
                        gao = wk.tile([128, 32], FP32, tag="gao")
                        acg = wk.tile([128, 32], FP32, tag="acg")
                        acif = wk.tile([128, 64], FP32, tag="acif")
                        aco = wk.tile([128, 32], FP32, tag="aco")
                        nc.vector.tensor_add(
                            out=gag[:, :], in0=ps[0][:, :],
                            in1=gxt[0][:, t * 32:(t + 1) * 32])
                        nc.scalar.activation(out=acg[:, :], in_=gag[:, :], func=AF.Tanh)
                        nc.vector.tensor_add(
                            out=gaif[:, 0:32], in0=ps[1][:, :],
                            in1=gxt[1][:, t * 32:(t + 1) * 32])
                        nc.vector.tensor_add(
                            out=gaif[:, 32:64], in0=ps[2][:, :],
                            in1=gxt[2][:, t * 32:(t + 1) * 32])
                        nc.scalar.activation(out=acif[:, :], in_=gaif[:, :], func=AF.Sigmoid)
                        nc.vector.tensor_add(
                            out=gao[:, :], in0=ps[3][:, :],
                            in1=gxt[3][:, t * 32:(t + 1) * 32])
                        nc.scalar.activation(out=aco[:, :], in_=gao[:, :], func=AF.Sigmoid)
                        ig = wk.tile([128, 32], FP32, tag="ig")
                        fc = wk.tile([128, 32], FP32, tag="fc")
                        tc_ = wk.tile([128, 32], FP32, tag="tc")
                        nc.vector.tensor_mul(out=ig[:, :], in0=acif[:, 0:32], in1=acg[:, :])
                        nc.vector.tensor_mul(out=fc[:, :], in0=acif[:, 32:64], in1=c_st[:, :])
                        nc.vector.tensor_add(out=c_st[:, :], in0=fc[:, :], in1=ig[:, :])
                        nc.scalar.activation(out=tc_[:, :], in_=c_st[:, :], func=AF.Tanh)
                        nc.vector.tensor_mul(out=h_st[:, :], in0=aco[:, :], in1=tc_[:, :])
                        nc.gpsimd.tensor_copy(out=ob[:, t * 32:(t + 1) * 32], in_=h_st[:, :])
                    nc.sync.dma_start(out=out_d[:, bass.ds(i0, T), :, :], in_=ob[:, :])
    if _KLDWOPT:
        _patch_walrus_flags()
        _refuse_ldweights(nc)
    _split_multiwaits(nc)
    return nc


def _prep_core_inputs(x_core, W, b):
    """x_core [BL, 256, S] f32 -> per-core input dict."""
    Wm = W[:, :, 1][GPERM]              # [2048, 768] reordered rows
    Wx = Wm[:, :CIN]                    # [2048, 256]
    Wh = Wm[:, CIN:]                    # [2048, 512]
    whT = np.ascontiguousarray(
        Wh.T.reshape(4, 128, G4).transpose(1, 0, 2)).astype(bfloat16)
    wxT = np.ascontiguousarray(
        Wx.T.reshape(2, 128, G4).transpose(1, 0, 2)).astype(bfloat16)
    bias = np.ascontiguousarray(b[GPERM].reshape(16, 128).T).astype(np.float32)
    # x_d [128 p, 2 kc, S, BL]: x_core[b, kc*128+p, s]
    xr = np.ascontiguousarray(
        x_core.reshape(BL, 2, 128, S).transpose(2, 1, 3, 0)).astype(bfloat16)
    return {"x": xr, "whT": whT, "wxT": wxT, "bias": bias}


def kernel(x, W, b):
    x = np.asarray(x, dtype=np.float32)
    W = np.asarray(W, dtype=np.float32)
    b = np.asarray(b, dtype=np.float32)
    nc = build_nc()
    in_maps = [_prep_core_inputs(x[c * BL:(c + 1) * BL], W, b)
               for c in range(NCORES)]
    res = bass_utils.run_bass_kernel_spmd(nc, in_maps, core_ids=list(range(NCORES)))
    outs = []
    for c in range(NCORES):
        o = np.asarray(res.results[c]["out"], dtype=np.float32)  # [128, S, 4, BL]
        outs.append(o.transpose(3, 2, 0, 1).reshape(BL, HC, S))
    return np.concatenate(outs, axis=0)


if __name__ == "__main__":
    d = np.load("/root/problem/ref_cache.npz")
    out = kernel(d["x"], d["W"], d["b"])
    exp = d["expected"]
    err = np.abs(out - exp).max() / (np.abs(exp).max() + 1e-9)
    print("rel err:", err)


# revision 12
# speedup vs baseline: 2.6001x; 1.0090x over previous
"""ConvLSTM (reduces to plain LSTM: conv over length-1 axis -> only middle tap).

Strategy: data-parallel over batch across 8 NeuronCores (B_local = 8/core).
  Phase 1 (bulk, parallel over time): gates_x = Wx @ x + b for all steps,
          stored bf16 in DRAM, gate-major-transposed layout.
  Phase 2 (sequential scan over S=2048): per step the recurrent matmul
          Wh @ h (bf16 weights stationary, h moving, N=8), gate adds on DVE,
          sigmoid/tanh on ACT, cell math on DVE; h written bf16 to DRAM.

Layouts (per core):
  Gate rows reordered to [g, i, f, o] blocks of 512 (ref order i,f,o,g).
  M-chunk m in 0..15: reordered gate rows m*128..m*128+127 (gamma = m//4, j = m%4).
  hidden unit u = 128*q + p lives at partition p, free-slot q.
  h/c state tiles: [128, 32] with col = q*8 + b_local.
"""

import sys
import numpy as np

for _p in ("/opt/trn_rl_repo",):
    if _p not in sys.path:
        sys.path.append(_p)

import concourse.bass as bass
import concourse.mybir as mybir
from concourse.tile import TileContext
from concourse import bass_utils
from ml_dtypes import bfloat16

AF = mybir.ActivationFunctionType
FP32 = mybir.dt.float32
BF16 = mybir.dt.bfloat16

B, CIN, S, HC = 64, 256, 2048, 512
NCORES = 8
BL = B // NCORES          # 8 batch per core
G4 = 4 * HC               # 2048 gate rows
T = 128                   # steps per For_i block
NBLK = S // T
NTOK = BL * S             # 16384 tokens per core
TOKB = 512                # tokens per precompute matmul
NTB = NTOK // TOKB        # 32 token blocks
# ref gate row order [i, f, o, g]; ours [g, i, f, o]
GPERM = np.concatenate([np.arange(1536, 2048), np.arange(0, 512),
                        np.arange(512, 1024), np.arange(1024, 1536)])


_KLDWOPT = False


def _patch_walrus_flags():
    """Enable walrus LDW optimization (background weight buffer) - requires
    self-loading matmuls (no standalone InstLdweights)."""
    if not _KLDWOPT:
        return
    import concourse.bass_utils as _bu
    if getattr(_bu.run_command, "_ldwopt_patched", False):
        return
    _orig = _bu.run_command

    def _run(cmd, **kw):
        cmd = ["--enable-ldw-opt=true" if c == "--enable-ldw-opt=false" else c
               for c in cmd]
        return _orig(cmd, **kw)

    _run._ldwopt_patched = True
    _bu.run_command = _run


def _refuse_ldweights(nc):
    """Fold each standalone InstLdweights into its following InstMatmult
    (self-loading matmul), merging sync waits/updates."""
    for f in nc.m.functions:
        for blk in f.blocks:
            newl = []
            pending = None
            for inst in blk.instructions:
                tn = type(inst).__name__
                if tn == "InstLdweights":
                    assert pending is None
                    pending = inst
                    continue
                if tn == "InstMatmult" and pending is not None:
                    lw = list(pending.sync_info.on_wait) if pending.sync_info else []
                    lu = list(pending.sync_info.on_update) if pending.sync_info else []
                    mw = list(inst.sync_info.on_wait) if inst.sync_info else []
                    mu = list(inst.sync_info.on_update) if inst.sync_info else []
                    inst.sync_info = mybir.SyncInfo(on_wait=lw + mw, on_update=lu + mu)
                    inst.ldweights = True
                    pending = None
                newl.append(inst)
            assert pending is None, "trailing InstLdweights without matmul"
            blk.instructions = newl


def _split_multiwaits(nc):
    """This walrus build allows only ONE sync-wait command per instruction.
    Hoist extra waits onto single-wait NoOps on the same engine stream."""
    nnop = 0
    for f in nc.m.functions:
        for blk in f.blocks:
            newl = []
            dirty = False
            for inst in blk.instructions:
                si = inst.sync_info
                if si and si.on_wait and len(si.on_wait) > 1:
                    waits = list(si.on_wait)
                    for w in waits[:-1]:
                        nop = mybir.InstNoOp(name=f"wsplit-{nnop}")
                        nnop += 1
                        nop.engine = inst.engine
                        nop.sync_info = mybir.SyncInfo(on_wait=[w], on_update=[])
                        newl.append(nop)
                    inst.sync_info = mybir.SyncInfo(
                        on_wait=[waits[-1]], on_update=list(si.on_update))
                    dirty = True
                newl.append(inst)
            if dirty:
                blk.instructions = newl
    return nnop


def build_nc():
    nc = bass.Bass()
    x_d = nc.dram_tensor("x", [128, 2, S, BL], BF16, kind="ExternalInput")
    whT_d = nc.dram_tensor("whT", [128, 4, G4], BF16, kind="ExternalInput")
    wxT_d = nc.dram_tensor("wxT", [128, 2, G4], BF16, kind="ExternalInput")
    b_d = nc.dram_tensor("bias", [128, 16], FP32, kind="ExternalInput")
    gx_d = nc.dram_tensor("gx", [4, 128, S, 4, BL], BF16, kind="Internal")
    out_d = nc.dram_tensor("out", [128, S, 4, BL], BF16, kind="ExternalOutput")

    with TileContext(nc) as tc:
        with (
            tc.tile_pool(name="const", bufs=1) as cpool,
            tc.tile_pool(name="state", bufs=1) as spool,
        ):
            whT = cpool.tile([128, 4 * G4], BF16)
            wxT = cpool.tile([128, 2 * G4], BF16)
            bias = cpool.tile([128, 16], FP32)
            nc.sync.dma_start(out=whT[:, :], in_=whT_d[:, :, :])
            nc.sync.dma_start(out=wxT[:, :], in_=wxT_d[:, :, :])
            nc.sync.dma_start(out=bias[:, :], in_=b_d[:, :])

            h_st = spool.tile([128, 32], BF16)
            gc = spool.tile([128, 64], FP32)  # [tanh_g | c] side by side
            nc.vector.memset(h_st[:, :], 0.0)
            nc.vector.memset(gc[:, :], 0.0)

            # ---------------- Phase 1: gates_x precompute ----------------
            with (
                tc.tile_pool(name="xin", bufs=3) as xpool,
                tc.tile_pool(name="pcps", bufs=4, space="PSUM") as pcps,
                tc.tile_pool(name="gxe", bufs=4) as gxep,
            ):
                TS = TOKB // BL  # 64 steps per token block
                for tb in range(NTB):
                    t0 = tb * TS
                    xt = [xpool.tile([128, TOKB], BF16, tag=f"x{k}", name=f"xt{k}") for k in range(2)]
                    for k in range(2):
                        nc.sync.dma_start(
                            out=xt[k][:, :],
                            in_=x_d[:, k, t0:t0 + TS, :])
                    for g in range(4):
                        # stage all 4 j-chunks of gate g in (t, j, b) order so
                        # the DRAM write is one fully-contiguous burst per row
                        ge = gxep.tile([128, 4 * TOKB], BF16, tag="ge")
                        gev = ge.rearrange("p (t j b) -> p t j b", t=TS, j=4, b=BL)
                        for j in range(4):
                            m = g * 4 + j
                            ps = pcps.tile([128, TOKB], FP32, tag="pc")
                            for k in range(2):
                                nc.tensor.matmul(
                                    ps[:, :],
                                    wxT[:, k * G4 + m * 128: k * G4 + (m + 1) * 128],
                                    xt[k][:, :],
                                    start=(k == 0), stop=(k == 1))
                            nc.scalar.activation(
                                out=gev[:, :, j, :], in_=ps[:, :], func=AF.Identity,
                                bias=bias[:, m:m + 1])
                        nc.gpsimd.dma_start(
                            out=gx_d[g, :, t0:t0 + TS, :, :],
                            in_=ge[:, :])

            # DRAM (gx_d) RAW across phases is not tracked by Tile -> hard barrier
            tc.strict_bb_all_engine_barrier()

            # ---------------- Phase 2: recurrence ----------------
            with (
                tc.tile_pool(name="gxin", bufs=2) as gxp,
                tc.tile_pool(name="obuf", bufs=2) as obp,
                tc.tile_pool(name="rps", bufs=2, space="PSUM") as rps,
                tc.tile_pool(name="work", bufs=6) as wk,
            ):
                with tc.For_i(0, S, T) as i0:
                    TH = T // 2
                    gxt = [[gxp.tile([128, TH * 32], BF16, tag=f"gx{g}h{h}",
                                     name=f"gxt{g}h{h}") for h in range(2)]
                           for g in range(4)]
                    for h in range(2):
                        for g in range(4):
                            nc.sync.dma_start(
                                out=gxt[g][h][:, :],
                                in_=gx_d[g, :, bass.ds(i0 + h * TH, TH), :, :])
                    ob = obp.tile([128, T * 32], BF16, tag="ob")

                    def gxs(g, t):
                        return gxt[g][t // TH][:, (t % TH) * 32:(t % TH + 1) * 32]

                    for t in range(T):
                        ps = [rps.tile([128, 32], FP32, tag=f"ps{g}", name=f"ps{g}") for g in range(4)]
                        for g in range(4):
                            for j in range(4):
                                m = g * 4 + j
                                for k in range(4):
                                    nc.tensor.matmul(
                                        ps[g][:, j * 8:(j + 1) * 8],
                                        whT[:, k * G4 + m * 128: k * G4 + (m + 1) * 128],
                                        h_st[:, k * 8:(k + 1) * 8],
                                        start=(k == 0), stop=(k == 3))
                        # gate order: 0=g 1=i 2=f 3=o ; i&f share a tile so
                        # one ACT sigmoid covers both
                        gag = wk.tile([128, 32], FP32, tag="gag")
                        gaif = wk.tile([128, 64], FP32, tag="gaif")
                        gao = wk.tile([128, 32], FP32, tag="gao")
                        acif = wk.tile([128, 64], FP32, tag="acif")
                        aco = wk.tile([128, 32], FP32, tag="aco")
                        nc.vector.tensor_add(
                            out=gag[:, :], in0=ps[0][:, :],
                            in1=gxs(0, t))
                        nc.scalar.activation(out=gc[:, 0:32], in_=gag[:, :], func=AF.Tanh)
                        nc.vector.tensor_add(
                            out=gaif[:, 0:32], in0=ps[1][:, :],
                            in1=gxs(1, t))
                        nc.vector.tensor_add(
                            out=gaif[:, 32:64], in0=ps[2][:, :],
                            in1=gxs(2, t))
                        nc.scalar.activation(out=acif[:, :], in_=gaif[:, :], func=AF.Sigmoid)
                        nc.vector.tensor_add(
                            out=gao[:, :], in0=ps[3][:, :],
                            in1=gxs(3, t))
                        nc.scalar.activation(out=aco[:, :], in_=gao[:, :], func=AF.Sigmoid)
                        igfc = wk.tile([128, 64], FP32, tag="igfc")
                        tc_ = wk.tile([128, 32], FP32, tag="tc")
                        nc.vector.tensor_mul(out=igfc[:, :], in0=acif[:, :], in1=gc[:, :])
                        nc.vector.tensor_add(out=gc[:, 32:64], in0=igfc[:, 0:32],
                                             in1=igfc[:, 32:64])
                        nc.scalar.activation(out=tc_[:, :], in_=gc[:, 32:64], func=AF.Tanh)
                        nc.vector.tensor_mul(out=h_st[:, :], in0=aco[:, :], in1=tc_[:, :])
                        nc.gpsimd.tensor_copy(out=ob[:, t * 32:(t + 1) * 32], in_=h_st[:, :])
                    nc.sync.dma_start(out=out_d[:, bass.ds(i0, T), :, :], in_=ob[:, :])
    if _KLDWOPT:
        _patch_walrus_flags()
        _refuse_ldweights(nc)
    _split_multiwaits(nc)
    return nc


def _prep_core_inputs(x_core, W, b):
    """x_core [BL, 256, S] f32 -> per-core input dict."""
    Wm = W[:, :, 1][GPERM]              # [2048, 768] reordered rows
    Wx = Wm[:, :CIN]                    # [2048, 256]
    Wh = Wm[:, CIN:]                    # [2048, 512]
    whT = np.ascontiguousarray(
        Wh.T.reshape(4, 128, G4).transpose(1, 0, 2)).astype(bfloat16)
    wxT = np.ascontiguousarray(
        Wx.T.reshape(2, 128, G4).transpose(1, 0, 2)).astype(bfloat16)
    bias = np.ascontiguousarray(b[GPERM].reshape(16, 128).T).astype(np.float32)
    # x_d [128 p, 2 kc, S, BL]: x_core[b, kc*128+p, s]
    xr = np.ascontiguousarray(
        x_core.reshape(BL, 2, 128, S).transpose(2, 1, 3, 0)).astype(bfloat16)
    return {"x": xr, "whT": whT, "wxT": wxT, "bias": bias}


def kernel(x, W, b):
    x = np.asarray(x, dtype=np.float32)
    W = np.asarray(W, dtype=np.float32)
    b = np.asarray(b, dtype=np.float32)
    nc = build_nc()
    in_maps = [_prep_core_inputs(x[c * BL:(c + 1) * BL], W, b)
               for c in range(NCORES)]
    res = bass_utils.run_bass_kernel_spmd(nc, in_maps, core_ids=list(range(NCORES)))
    outs = []
    for c in range(NCORES):
        o = np.asarray(res.results[c]["out"], dtype=np.float32)  # [128, S, 4, BL]
        outs.append(o.transpose(3, 2, 0, 1).reshape(BL, HC, S))
    return np.concatenate(outs, axis=0)


if __name__ == "__main__":
    d = np.load("/root/problem/ref_cache.npz")
    out = kernel(d["x"], d["W"], d["b"])
    exp = d["expected"]
    err = np.abs(out - exp).max() / (np.abs(exp).max() + 1e-9)
    print("rel err:", err)
